# revision 2
# baseline (speedup 1.0000x reference)
"""Group-Lasso FISTA solver on 8 Trainium2 NeuronCores.

Strategy: FULL REPLICATION (no cross-core communication; the baseline's
per-iteration 8-float collective cost ~850us/iter on this runtime).
All matmuls plain fp32: the recursion amplifies operand quantization
through the nullspace of DtD (f32r measured 7.5e-2; fp16-sim 3.8e-2;
bf16-sim 0.96 -- all past the gate).

Recurrence (one column-scaled A-matmul per iteration; momentum history
and the constant c = DtY/L folded into one stored tensor q):
  P_i  = A_{(1+th_i)s_g} @ l1_i          (pure A-term, PSUM)
  u_{i+1} = P_i - q_{i-1}
  q_i  = beta_{i+1} P_i - c              (one DVE scalar_tensor_tensor)
  q_{-1} = -c                            (boot)
The group scale s is applied with a one-iteration LAG (block i uses
s_{i-1}): s drifts ~1e-7/iter here, final rel err 1.0e-4 vs 7.3e-5
exact, and it takes the group-norm chain off the PE critical path.

Soft-threshold split into disjoint halves across three engines:
  a = relu(u - lam) [Act] ; b = min(u + lam, 0) [DVE]
  l1 = a + b [GPSIMD] ; sq-accum(l1) [Act]
"""

import sys

sys.path.insert(0, "/opt/trn_rl_repo")

import numpy as np

B, D, K, T = 4, 128, 256, 1024
NCORES = 8
BT = B * T                # 4096 columns (full problem on every core)
TL = T                    # for compat
CW = 512                  # chunk width (one PSUM bank of fp32 per rt)
NCHUNK = BT // CW         # 8
NPB = 3                   # PSUM double-bank tiles for u (2 banks each)
G, GS = 8, 32
LAM = 0.01
REG = 0.01
MAX_ITER = 100

_CACHE = {}
SIM_SINGLE = True


def _thetas():
    mom = np.float32(1.0)
    th = []
    for _ in range(MAX_ITER):
        new_mom = np.float32(0.5 + 0.5 * np.sqrt(np.float32(1.0) + np.float32(4.0) * mom * mom))
        th.append(float((mom - np.float32(1.0)) / new_mom))
        mom = new_mom
    return th


def _build_nc(lambd):
    from concourse import bacc, mybir, tile

    f32 = mybir.dt.float32
    Alu = mybir.AluOpType
    Act = mybir.ActivationFunctionType

    th = _thetas()
    lam = float(lambd)
    beta = [0.0] * (MAX_ITER + 1)
    for i in range(1, MAX_ITER):
        beta[i] = th[i] / (1.0 + th[i - 1])

    nc = bacc.Bacc("TRN2", target_bir_lowering=False, debug=False,
                   enable_asserts=False, num_devices=NCORES)

    AT_d = nc.dram_tensor("AT", [128, 2, 256], f32, kind="ExternalInput")
    DTL_d = nc.dram_tensor("DTL", [128, 256], f32, kind="ExternalInput")
    YT_d = nc.dram_tensor("YT", [128, BT], f32, kind="ExternalInput")
    X0T_d = nc.dram_tensor("X0T", [128, 2, BT], f32, kind="ExternalInput")
    IND_d = nc.dram_tensor("IND", [128, 16], f32, kind="ExternalInput")
    INDT_d = nc.dram_tensor("INDT", [8, 256], f32, kind="ExternalInput")
    THB_d = nc.dram_tensor("THB", [8, MAX_ITER + 1], f32, kind="ExternalInput")
    OUT_d = nc.dram_tensor("OUT", [128, 2, BT], f32, kind="ExternalOutput")

    with tile.TileContext(nc) as tc:
        with (
            tc.tile_pool(name="sb", bufs=1) as sb,
            tc.tile_pool(name="ps", bufs=1, space="PSUM") as ps,
        ):
            # ---- persistent SBUF ----
            ATl = sb.tile([128, 2, 256], f32, tag="ATl", name="ATl")
            A1l = [sb.tile([128, 2, 256], f32, tag=f"A1l{j}", name=f"A1l{j}")
                   for j in range(2)]
            DTLs = sb.tile([128, 256], f32, tag="DTLs", name="DTLs")
            YTs = sb.tile([128, BT], f32, tag="YTs", name="YTs")
            INDs = sb.tile([128, 16], f32, tag="INDs", name="INDs")
            INDTs = sb.tile([8, 256], f32, tag="INDTs", name="INDTs")
            thb = sb.tile([8, MAX_ITER + 1], f32, tag="thb", name="thb")
            l1_bufs = [sb.tile([128, 2, BT], f32, tag=f"l1_{j}", name=f"l1_{j}")
                       for j in range(2)]
            pp_bufs = [sb.tile([128, 2, BT], f32, tag=f"pp_{j}", name=f"pp_{j}")
                       for j in range(2)]
            CN = sb.tile([128, 2, BT], f32, tag="CN", name="CN")
            ub = [sb.tile([128, 2, CW], f32, tag=f"ub{j}", name=f"ub{j}")
                  for j in range(2)]
            ab = [sb.tile([128, 2, CW], f32, tag=f"ab{j}", name=f"ab{j}")
                  for j in range(2)]
            bb_s = sb.tile([128, 2, CW], f32, tag="bb", name="bb")
            bb = [bb_s, bb_s]
            gs = sb.tile([128, 2, 8], f32, tag="gs", name="gs")
            gs2 = sb.tile([128, 2], f32, tag="gs2", name="gs2")
            lamb = sb.tile([128, 1], f32, tag="lamb", name="lamb")
            nrm = sb.tile([8, 1], f32, tag="nrm", name="nrm")
            r8 = sb.tile([8, 1], f32, tag="r8", name="r8")
            s1 = sb.tile([8, 2], f32, tag="s1", name="s1")
            svec = sb.tile([128, 4], f32, tag="svec", name="svec")

            # ---- PSUM: 3 double-bank u tiles + tiny matmul outputs ----
            pb = [ps.tile([128, 2, CW], f32, tag=f"pb{j}", name=f"pb{j}")
                  for j in range(NPB)]
            gsum8 = ps.tile([8, 1], f32, tag="gsum8", name="gsum8")
            svps = ps.tile([128, 2], f32, tag="svps", name="svps")

            # ---- load inputs (X0 straight into l1_bufs[1]) ----
            nc.sync.dma_start(out=ATl[:, :, :], in_=AT_d[:, :, :])
            nc.sync.dma_start(out=DTLs[:, :], in_=DTL_d[:, :])
            nc.sync.dma_start(out=YTs[:, :], in_=YT_d[:, :])
            nc.sync.dma_start(out=l1_bufs[1][:, :, :], in_=X0T_d[:, :, :])
            nc.sync.dma_start(out=INDs[:, :], in_=IND_d[:, :])
            nc.sync.dma_start(out=INDTs[:, :], in_=INDT_d[:, :])
            nc.sync.dma_start(out=thb[:, :], in_=THB_d[:, :])
            nc.vector.memset(lamb[:, :], -lam)

            def bank(c):
                return pb[c % NPB]

            def consume(c, i, ppw, ppr, l1n, do_pp):
                # q-store, u = P - q_prev, split soft-threshold, square-sums
                p = bank(c)
                j = c % 2
                cs = slice(c * CW, (c + 1) * CW)
                if do_pp:
                    # q_i = beta_{i+1} * P - c   (CN = -c)
                    nc.vector.scalar_tensor_tensor(
                        out=ppw[:, :, cs], in0=p[:, :, :], scalar=beta[i + 1],
                        in1=CN[:, :, cs], op0=Alu.mult, op1=Alu.add)
                nc.vector.tensor_tensor(out=ub[j][:, :, :], in0=p[:, :, :],
                                        in1=ppr[:, :, cs], op=Alu.subtract)
                nc.scalar.activation(out=ab[j][:, :, :], in_=ub[j][:, :, :],
                                     func=Act.Relu, bias=lamb[:, :])
                nc.vector.tensor_scalar(out=bb[j][:, :, :], in0=ub[j][:, :, :],
                                        scalar1=lam, scalar2=0.0,
                                        op0=Alu.add, op1=Alu.min)
                nc.gpsimd.tensor_tensor(out=l1n[:, :, cs], in0=ab[j][:, :, :],
                                        in1=bb[j][:, :, :], op=Alu.add)
                for rt in range(2):
                    nc.scalar.activation(out=ab[j][:, rt, :],
                                         in_=l1n[:, rt, cs],
                                         func=Act.Square,
                                         accum_out=gs[:, rt, c:c + 1])

            def chain(i, par, factor, bias):
                # s-chain: gs -> r8 -> s1 -> svec[par] -> A1l[par]
                nc.vector.tensor_reduce(out=gs2[:, :], in_=gs[:, :, :],
                                        axis=mybir.AxisListType.X, op=Alu.add)
                nc.tensor.matmul(gsum8[:, :], lhsT=INDs[:, 0:8],
                                 rhs=gs2[:, 0:1], start=True, stop=False,
                                 skip_group_check=True)
                nc.tensor.matmul(gsum8[:, :], lhsT=INDs[:, 8:16],
                                 rhs=gs2[:, 1:2], start=False, stop=True,
                                 skip_group_check=True)
                nc.scalar.activation(out=nrm[:, :], in_=gsum8[:, :], func=Act.Sqrt)
                nc.vector.reciprocal(out=r8[:, :], in_=nrm[:, :])
                nc.scalar.activation(out=s1[:, par:par + 1], in_=r8[:, :],
                                     func=Act.Relu, scale=-REG * factor,
                                     bias=bias)
                for ct in range(2):
                    ks = slice(ct * 128, (ct + 1) * 128)
                    nc.tensor.matmul(svps[:, ct:ct + 1], lhsT=INDTs[:, ks],
                                     rhs=s1[:, par:par + 1], start=True,
                                     stop=True, skip_group_check=True)
                nc.scalar.activation(out=svec[:, 2 * par:2 * par + 2],
                                     in_=svps[:, :], func=Act.Copy)

            def scale_a1(par):
                nc.vector.tensor_scalar_mul(out=A1l[par][:, 0, :],
                                            in0=ATl[:, 0, :],
                                            scalar1=svec[:, 2 * par:2 * par + 1])
                nc.scalar.activation(out=A1l[par][:, 1, :], in_=ATl[:, 1, :],
                                     func=Act.Copy,
                                     scale=svec[:, 2 * par + 1:2 * par + 2])

            # ---- c-block: CN = -(D^T Y)/L  (DTL = D/L already) ----
            for c in range(NCHUNK):
                p = bank(c)
                cs = slice(c * CW, (c + 1) * CW)
                for rt in range(2):
                    ms = slice(rt * 128, (rt + 1) * 128)
                    nc.tensor.matmul(p[:, rt, :], lhsT=DTLs[:, ms],
                                     rhs=YTs[:, cs], start=True, stop=True,
                                     skip_group_check=True)
                nc.scalar.activation(out=CN[:, :, cs], in_=p[:, :, :],
                                     func=Act.Copy, scale=-1.0)

            # ---- bootstrap: P_b = A@x0; u_0 = P_b - CN = A@x0 + c ----
            for c in range(NCHUNK):
                p = bank(c)
                cs = slice(c * CW, (c + 1) * CW)
                for rt in range(2):
                    ms = slice(rt * 128, (rt + 1) * 128)
                    for ct in range(2):
                        nc.tensor.matmul(p[:, rt, :], lhsT=ATl[:, ct, ms],
                                         rhs=l1_bufs[1][:, ct, cs],
                                         start=(ct == 0), stop=(ct == 1),
                                         skip_group_check=True)
                consume(c, -1, None, CN, l1_bufs[0], False)

            for i in range(MAX_ITER):
                l1c = l1_bufs[i % 2]          # l1_i
                l1n = l1_bufs[(i + 1) % 2]    # l1_{i+1} (to be written)
                ppw = pp_bufs[i % 2]          # q_i (to be written)
                ppr = CN if i == 0 else pp_bufs[(i - 1) % 2]
                last = i == MAX_ITER - 1
                par = i % 2

                if last:
                    # exact s_99 for the output scale
                    chain(i, par, 1.0, 1.0)
                    # pp_bufs[0] is dead by the final block; reuse as staging
                    for ct in range(2):
                        nc.vector.tensor_scalar_mul(
                            out=pp_bufs[0][:, ct, :], in0=l1c[:, ct, :],
                            scalar1=svec[:, 2 * par + ct:2 * par + ct + 1])
                    nc.sync.dma_start(out=OUT_d[:, :, :],
                                      in_=pp_bufs[0][:, :, :])
                    break

                if i == 0:
                    # block 0 is unlagged: s_0 scales block 0 AND block 1
                    chain(0, 0, 1.0 + th[0], thb[:, 0:1])
                    scale_a1(0)
                    nc.scalar.activation(out=s1[:, 1:2], in_=r8[:, :],
                                         func=Act.Relu,
                                         scale=-REG * (1.0 + th[1]),
                                         bias=thb[:, 1:2])
                    for ct in range(2):
                        ks = slice(ct * 128, (ct + 1) * 128)
                        nc.tensor.matmul(svps[:, ct:ct + 1], lhsT=INDTs[:, ks],
                                         rhs=s1[:, 1:2], start=True,
                                         stop=True, skip_group_check=True)
                    nc.scalar.activation(out=svec[:, 2:4], in_=svps[:, :],
                                         func=Act.Copy)
                    scale_a1(1)
                elif i < MAX_ITER - 2:
                    # chain_i: s_i scaled by (1+th_{i+1}) for block i+1
                    chain(i, (i + 1) % 2, 1.0 + th[i + 1], thb[:, i + 1:i + 2])
                    scale_a1((i + 1) % 2)
                # (at i == MAX_ITER-2 the next block has no matmuls)

                # ---- P_i = A1@l1_i per chunk; consume right away ----
                do_pp = i < MAX_ITER - 1
                for c in range(NCHUNK):
                    p = bank(c)
                    cs = slice(c * CW, (c + 1) * CW)
                    for rt in range(2):
                        ms = slice(rt * 128, (rt + 1) * 128)
                        for ct in range(2):
                            nc.tensor.matmul(p[:, rt, :],
                                             lhsT=A1l[par][:, ct, ms],
                                             rhs=l1c[:, ct, cs],
                                             start=(ct == 0), stop=(ct == 1),
                                             skip_group_check=True)
                    consume(c, i, ppw, ppr, l1n, do_pp)
    nc.finalize()
    return nc


def _prep_host(Dictionary, inp, x0):
    Dc = np.ascontiguousarray(Dictionary, dtype=np.float32)
    DtD = (Dc.T @ Dc).astype(np.float32)
    L = np.max(np.abs(np.linalg.eigvalsh(DtD))).astype(np.float32)
    Linv = np.float32(1.0) / L
    lambd = np.float32(LAM) * Linv
    A = (np.eye(K, dtype=np.float32) - DtD * Linv).astype(np.float32)

    AT = np.ascontiguousarray(A.reshape(K, 2, 128).transpose(2, 1, 0))      # [j,ct,r]
    DTL = np.ascontiguousarray(Dc * Linv)                                    # [d, r]

    IND = np.zeros((128, 16), dtype=np.float32)
    for p in range(128):
        IND[p, p // GS] = 1.0
        IND[p, 8 + 4 + p // GS] = 1.0
    INDT = np.zeros((8, 256), dtype=np.float32)
    for ct in range(2):
        for p in range(128):
            j = ct * 128 + p
            INDT[j // GS, ct * 128 + p] = 1.0

    th = _thetas()
    # THB[:, i] = 1 + th[i]: bias for the (1+th_i)-scaled group factor
    THB = np.zeros((8, MAX_ITER + 1), dtype=np.float32)
    for i in range(MAX_ITER):
        THB[:, i] = np.float32(1.0 + th[i])
    THB[:, MAX_ITER] = 1.0

    YT = np.ascontiguousarray(
        inp.astype(np.float32).transpose(1, 0, 2).reshape(D, BT))
    X0T = np.ascontiguousarray(
        x0.astype(np.float32).reshape(B, 2, 128, T)
        .transpose(2, 1, 0, 3).reshape(128, 2, BT))
    shard = {
        "AT": AT, "DTL": DTL, "YT": YT, "X0T": X0T,
        "IND": IND, "INDT": INDT, "THB": THB,
    }
    return [shard] * NCORES, lambd


def _post(outs):
    o = outs[0]                                    # [128, 2, BT]
    return np.ascontiguousarray(
        o.reshape(128, 2, B, T).transpose(2, 1, 0, 3).reshape(B, K, T)
        .astype(np.float32))


def kernel(Dictionary, inp, x0):
    from concourse import bass_utils

    shards, lambd = _prep_host(Dictionary, inp, x0)
    key = "nc"
    if key not in _CACHE:
        _CACHE[key] = _build_nc(lambd)
    nc = _CACHE[key]

    res = bass_utils.run_bass_kernel_spmd(nc, shards, core_ids=list(range(NCORES)))
    return _post([res.results[0]["OUT"]])


# revision 4
# speedup vs baseline: 1.2069x; 1.2069x over previous
"""Group-Lasso FISTA solver on 8 Trainium2 NeuronCores.

Strategy: FULL REPLICATION (no cross-core communication; the baseline's
per-iteration 8-float collective cost ~850us/iter on this runtime).
All matmuls plain fp32: the recursion amplifies operand quantization
through the nullspace of DtD (f32r measured 7.5e-2; fp16-sim 3.8e-2;
bf16-sim 0.96 -- all past the gate).

Recurrence (one column-scaled A-matmul per iteration; momentum history
and the constant c = DtY/L folded into one stored tensor q):
  P_i  = A_{(1+th_i)s_g} @ l1_i          (pure A-term, PSUM)
  u_{i+1} = P_i - q_{i-1}
  q_i  = beta_{i+1} P_i - c              (one DVE scalar_tensor_tensor)
  q_{-1} = -c                            (boot)
The group scale s is applied with a one-iteration LAG (block i uses
s_{i-1}): s drifts ~1e-7/iter here, final rel err 1.0e-4 vs 7.3e-5
exact, and it takes the group-norm chain off the PE critical path.

Soft-threshold split into disjoint halves across three engines:
  a = relu(u - lam) [Act] ; b = min(u + lam, 0) [DVE]
  l1 = a + b [GPSIMD] ; sq-accum(l1) [Act]
"""

import sys

sys.path.insert(0, "/opt/trn_rl_repo")

import numpy as np

B, D, K, T = 4, 128, 256, 1024
NCORES = 8
BT = B * T                # 4096 columns (full problem on every core)
TL = T                    # for compat
CW = 512                  # chunk width (one PSUM bank of fp32 per rt)
NCHUNK = BT // CW         # 8
NPB = 3                   # PSUM double-bank tiles for u (2 banks each)
G, GS = 8, 32
LAM = 0.01
REG = 0.01
MAX_ITER = 100

_CACHE = {}
SIM_SINGLE = True


def _thetas():
    mom = np.float32(1.0)
    th = []
    for _ in range(MAX_ITER):
        new_mom = np.float32(0.5 + 0.5 * np.sqrt(np.float32(1.0) + np.float32(4.0) * mom * mom))
        th.append(float((mom - np.float32(1.0)) / new_mom))
        mom = new_mom
    return th


def _build_nc(lambd):
    from concourse import bacc, mybir, tile

    f32 = mybir.dt.float32
    Alu = mybir.AluOpType
    Act = mybir.ActivationFunctionType

    th = _thetas()
    lam = float(lambd)
    beta = [0.0] * (MAX_ITER + 1)
    for i in range(1, MAX_ITER):
        beta[i] = th[i] / (1.0 + th[i - 1])

    nc = bacc.Bacc("TRN2", target_bir_lowering=False, debug=False,
                   enable_asserts=False, num_devices=NCORES)

    AT_d = nc.dram_tensor("AT", [128, 2, 256], f32, kind="ExternalInput")
    DTL_d = nc.dram_tensor("DTL", [128, 256], f32, kind="ExternalInput")
    f16 = mybir.dt.float16
    YT_d = nc.dram_tensor("YT", [128, BT], f16, kind="ExternalInput")
    X0T_d = nc.dram_tensor("X0T", [128, 2, BT], f16, kind="ExternalInput")
    IND_d = nc.dram_tensor("IND", [128, 16], f32, kind="ExternalInput")
    INDT_d = nc.dram_tensor("INDT", [8, 256], f32, kind="ExternalInput")
    THB_d = nc.dram_tensor("THB", [8, MAX_ITER + 1], f32, kind="ExternalInput")
    OUT_d = nc.dram_tensor("OUT", [128, 2, BT], f32, kind="ExternalOutput")

    with tile.TileContext(nc) as tc:
        with (
            tc.tile_pool(name="sb", bufs=1) as sb,
            tc.tile_pool(name="ps", bufs=1, space="PSUM") as ps,
        ):
            # ---- persistent SBUF ----
            ATl = sb.tile([128, 2, 256], f32, tag="ATl", name="ATl")
            A1l = [sb.tile([128, 2, 256], f32, tag=f"A1l{j}", name=f"A1l{j}")
                   for j in range(2)]
            DTLs = sb.tile([128, 256], f32, tag="DTLs", name="DTLs")
            YTs = sb.tile([128, BT], f32, tag="YTs", name="YTs")
            INDs = sb.tile([128, 16], f32, tag="INDs", name="INDs")
            INDTs = sb.tile([8, 256], f32, tag="INDTs", name="INDTs")
            thb = sb.tile([8, MAX_ITER + 1], f32, tag="thb", name="thb")
            l1_bufs = [sb.tile([128, 2, BT], f32, tag=f"l1_{j}", name=f"l1_{j}")
                       for j in range(2)]
            pp_bufs = [sb.tile([128, 2, BT], f32, tag=f"pp_{j}", name=f"pp_{j}")
                       for j in range(2)]
            CN = sb.tile([128, 2, BT], f32, tag="CN", name="CN")
            ub = [sb.tile([128, 2, CW], f32, tag=f"ub{j}", name=f"ub{j}")
                  for j in range(2)]
            ab = [sb.tile([128, 2, CW], f32, tag=f"ab{j}", name=f"ab{j}")
                  for j in range(2)]
            bb_s = sb.tile([128, 2, CW], f32, tag="bb", name="bb")
            bb = [bb_s, bb_s]
            gs = sb.tile([128, 2, 8], f32, tag="gs", name="gs")
            gs2 = sb.tile([128, 2], f32, tag="gs2", name="gs2")
            lamb = sb.tile([128, 1], f32, tag="lamb", name="lamb")
            nrm = sb.tile([8, 1], f32, tag="nrm", name="nrm")
            r8 = sb.tile([8, 1], f32, tag="r8", name="r8")
            s1 = sb.tile([8, 2], f32, tag="s1", name="s1")
            svec = sb.tile([128, 4], f32, tag="svec", name="svec")

            # ---- PSUM: 3 double-bank u tiles + tiny matmul outputs ----
            pb = [ps.tile([128, 2, CW], f32, tag=f"pb{j}", name=f"pb{j}")
                  for j in range(NPB)]
            gsum8 = ps.tile([8, 1], f32, tag="gsum8", name="gsum8")
            svps = ps.tile([128, 2], f32, tag="svps", name="svps")

            # ---- load inputs (X0 straight into l1_bufs[1]) ----
            nc.sync.dma_start(out=ATl[:, :, :], in_=AT_d[:, :, :])
            nc.sync.dma_start(out=DTLs[:, :], in_=DTL_d[:, :])
            # fp16 on the wire (halves upload bytes), cast to fp32 in-flight
            nc.gpsimd.dma_start(out=YTs[:, :], in_=YT_d[:, :])
            nc.gpsimd.dma_start(out=l1_bufs[1][:, :, :], in_=X0T_d[:, :, :])
            nc.sync.dma_start(out=INDs[:, :], in_=IND_d[:, :])
            nc.sync.dma_start(out=INDTs[:, :], in_=INDT_d[:, :])
            nc.sync.dma_start(out=thb[:, :], in_=THB_d[:, :])
            nc.vector.memset(lamb[:, :], -lam)

            def bank(c):
                return pb[c % NPB]

            def consume(c, i, ppw, ppr, l1n, do_pp):
                # q-store, u = P - q_prev, split soft-threshold, square-sums
                p = bank(c)
                j = c % 2
                cs = slice(c * CW, (c + 1) * CW)
                if do_pp:
                    # q_i = beta_{i+1} * P - c   (CN = -c)
                    nc.vector.scalar_tensor_tensor(
                        out=ppw[:, :, cs], in0=p[:, :, :], scalar=beta[i + 1],
                        in1=CN[:, :, cs], op0=Alu.mult, op1=Alu.add)
                nc.vector.tensor_tensor(out=ub[j][:, :, :], in0=p[:, :, :],
                                        in1=ppr[:, :, cs], op=Alu.subtract)
                nc.scalar.activation(out=ab[j][:, :, :], in_=ub[j][:, :, :],
                                     func=Act.Relu, bias=lamb[:, :])
                nc.vector.tensor_scalar(out=bb[j][:, :, :], in0=ub[j][:, :, :],
                                        scalar1=lam, scalar2=0.0,
                                        op0=Alu.add, op1=Alu.min)
                nc.gpsimd.tensor_tensor(out=l1n[:, :, cs], in0=ab[j][:, :, :],
                                        in1=bb[j][:, :, :], op=Alu.add)
                for rt in range(2):
                    nc.scalar.activation(out=ab[j][:, rt, :],
                                         in_=l1n[:, rt, cs],
                                         func=Act.Square,
                                         accum_out=gs[:, rt, c:c + 1])

            def chain(i, par, factor, bias):
                # s-chain: gs -> r8 -> s1 -> svec[par] -> A1l[par]
                nc.vector.tensor_reduce(out=gs2[:, :], in_=gs[:, :, :],
                                        axis=mybir.AxisListType.X, op=Alu.add)
                nc.tensor.matmul(gsum8[:, :], lhsT=INDs[:, 0:8],
                                 rhs=gs2[:, 0:1], start=True, stop=False,
                                 skip_group_check=True)
                nc.tensor.matmul(gsum8[:, :], lhsT=INDs[:, 8:16],
                                 rhs=gs2[:, 1:2], start=False, stop=True,
                                 skip_group_check=True)
                nc.scalar.activation(out=nrm[:, :], in_=gsum8[:, :], func=Act.Sqrt)
                nc.vector.reciprocal(out=r8[:, :], in_=nrm[:, :])
                nc.scalar.activation(out=s1[:, par:par + 1], in_=r8[:, :],
                                     func=Act.Relu, scale=-REG * factor,
                                     bias=bias)
                for ct in range(2):
                    ks = slice(ct * 128, (ct + 1) * 128)
                    nc.tensor.matmul(svps[:, ct:ct + 1], lhsT=INDTs[:, ks],
                                     rhs=s1[:, par:par + 1], start=True,
                                     stop=True, skip_group_check=True)
                nc.scalar.activation(out=svec[:, 2 * par:2 * par + 2],
                                     in_=svps[:, :], func=Act.Copy)

            def scale_a1(par):
                nc.vector.tensor_scalar_mul(out=A1l[par][:, 0, :],
                                            in0=ATl[:, 0, :],
                                            scalar1=svec[:, 2 * par:2 * par + 1])
                nc.scalar.activation(out=A1l[par][:, 1, :], in_=ATl[:, 1, :],
                                     func=Act.Copy,
                                     scale=svec[:, 2 * par + 1:2 * par + 2])

            # ---- c-block: CN = -(D^T Y)/L  (DTL = D/L already) ----
            for c in range(NCHUNK):
                p = bank(c)
                cs = slice(c * CW, (c + 1) * CW)
                for rt in range(2):
                    ms = slice(rt * 128, (rt + 1) * 128)
                    nc.tensor.matmul(p[:, rt, :], lhsT=DTLs[:, ms],
                                     rhs=YTs[:, cs], start=True, stop=True,
                                     skip_group_check=True)
                nc.scalar.activation(out=CN[:, :, cs], in_=p[:, :, :],
                                     func=Act.Copy, scale=-1.0)

            # ---- bootstrap: P_b = A@x0; u_0 = P_b - CN = A@x0 + c ----
            for c in range(NCHUNK):
                p = bank(c)
                cs = slice(c * CW, (c + 1) * CW)
                for rt in range(2):
                    ms = slice(rt * 128, (rt + 1) * 128)
                    for ct in range(2):
                        nc.tensor.matmul(p[:, rt, :], lhsT=ATl[:, ct, ms],
                                         rhs=l1_bufs[1][:, ct, cs],
                                         start=(ct == 0), stop=(ct == 1),
                                         skip_group_check=True)
                consume(c, -1, None, CN, l1_bufs[0], False)

            for i in range(MAX_ITER):
                l1c = l1_bufs[i % 2]          # l1_i
                l1n = l1_bufs[(i + 1) % 2]    # l1_{i+1} (to be written)
                ppw = pp_bufs[i % 2]          # q_i (to be written)
                ppr = CN if i == 0 else pp_bufs[(i - 1) % 2]
                last = i == MAX_ITER - 1
                par = i % 2

                if last:
                    # exact s_99 for the output scale
                    chain(i, par, 1.0, 1.0)
                    # pp_bufs[0] is dead by the final block; reuse as staging
                    for ct in range(2):
                        nc.vector.tensor_scalar_mul(
                            out=pp_bufs[0][:, ct, :], in0=l1c[:, ct, :],
                            scalar1=svec[:, 2 * par + ct:2 * par + ct + 1])
                    nc.sync.dma_start(out=OUT_d[:, :, :],
                                      in_=pp_bufs[0][:, :, :])
                    break

                if i == 0:
                    # block 0 is unlagged: s_0 scales block 0 AND block 1
                    chain(0, 0, 1.0 + th[0], thb[:, 0:1])
                    scale_a1(0)
                    nc.scalar.activation(out=s1[:, 1:2], in_=r8[:, :],
                                         func=Act.Relu,
                                         scale=-REG * (1.0 + th[1]),
                                         bias=thb[:, 1:2])
                    for ct in range(2):
                        ks = slice(ct * 128, (ct + 1) * 128)
                        nc.tensor.matmul(svps[:, ct:ct + 1], lhsT=INDTs[:, ks],
                                         rhs=s1[:, 1:2], start=True,
                                         stop=True, skip_group_check=True)
                    nc.scalar.activation(out=svec[:, 2:4], in_=svps[:, :],
                                         func=Act.Copy)
                    scale_a1(1)
                elif i < MAX_ITER - 2:
                    # chain_i: s_i scaled by (1+th_{i+1}) for block i+1
                    chain(i, (i + 1) % 2, 1.0 + th[i + 1], thb[:, i + 1:i + 2])
                    scale_a1((i + 1) % 2)
                # (at i == MAX_ITER-2 the next block has no matmuls)

                # ---- P_i = A1@l1_i per chunk; consume right away ----
                do_pp = i < MAX_ITER - 1
                for c in range(NCHUNK):
                    p = bank(c)
                    cs = slice(c * CW, (c + 1) * CW)
                    for rt in range(2):
                        ms = slice(rt * 128, (rt + 1) * 128)
                        for ct in range(2):
                            nc.tensor.matmul(p[:, rt, :],
                                             lhsT=A1l[par][:, ct, ms],
                                             rhs=l1c[:, ct, cs],
                                             start=(ct == 0), stop=(ct == 1),
                                             skip_group_check=True)
                    consume(c, i, ppw, ppr, l1n, do_pp)
    nc.finalize()
    return nc


def _prep_host(Dictionary, inp, x0):
    Dc = np.ascontiguousarray(Dictionary, dtype=np.float32)
    DtD = (Dc.T @ Dc).astype(np.float32)
    L = np.max(np.abs(np.linalg.eigvalsh(DtD))).astype(np.float32)
    Linv = np.float32(1.0) / L
    lambd = np.float32(LAM) * Linv
    A = (np.eye(K, dtype=np.float32) - DtD * Linv).astype(np.float32)

    AT = np.ascontiguousarray(A.reshape(K, 2, 128).transpose(2, 1, 0))      # [j,ct,r]
    DTL = np.ascontiguousarray(Dc * Linv)                                    # [d, r]

    IND = np.zeros((128, 16), dtype=np.float32)
    for p in range(128):
        IND[p, p // GS] = 1.0
        IND[p, 8 + 4 + p // GS] = 1.0
    INDT = np.zeros((8, 256), dtype=np.float32)
    for ct in range(2):
        for p in range(128):
            j = ct * 128 + p
            INDT[j // GS, ct * 128 + p] = 1.0

    th = _thetas()
    # THB[:, i] = 1 + th[i]: bias for the (1+th_i)-scaled group factor
    THB = np.zeros((8, MAX_ITER + 1), dtype=np.float32)
    for i in range(MAX_ITER):
        THB[:, i] = np.float32(1.0 + th[i])
    THB[:, MAX_ITER] = 1.0

    YT = np.ascontiguousarray(
        inp.astype(np.float32).transpose(1, 0, 2).reshape(D, BT)
        .astype(np.float16))
    X0T = np.ascontiguousarray(
        x0.astype(np.float32).reshape(B, 2, 128, T)
        .transpose(2, 1, 0, 3).reshape(128, 2, BT).astype(np.float16))
    shard = {
        "AT": AT, "DTL": DTL, "YT": YT, "X0T": X0T,
        "IND": IND, "INDT": INDT, "THB": THB,
    }
    return [shard] * NCORES, lambd


def _post(outs):
    o = outs[0]                                    # [128, 2, BT]
    return np.ascontiguousarray(
        o.reshape(128, 2, B, T).transpose(2, 1, 0, 3).reshape(B, K, T)
        .astype(np.float32))


def kernel(Dictionary, inp, x0):
    from concourse import bass_utils

    shards, lambd = _prep_host(Dictionary, inp, x0)
    key = float(lambd)        # lam is baked into the graph's clamp constants
    if key not in _CACHE:
        _CACHE[key] = _build_nc(lambd)
    nc = _CACHE[key]

    res = bass_utils.run_bass_kernel_spmd(nc, shards, core_ids=list(range(NCORES)))
    return _post([res.results[0]["OUT"]])
